# revision 2
# baseline (speedup 1.0000x reference)
"""VQ-codebook autoencoder Trainium2 kernel (v3).

Data-parallel over 8 NeuronCores: batch 1024 -> 8 x 128.

Per-core layout: feature rows on partitions, (t-block, batch) on the free
dim: F = NTB * B = 64 * 128 = 8192, free index = tb*128 + b, TB=8 taus per
t-block.  fp16 activations, fp32 psum.

Every engine access needs a 32-aligned partition base, so tau-row maps are
gapped to put halo sources/destinations at 32-aligned rows; the gaps live
inside matmul outputs as zero weight columns (no extra instruction cost):
  x3 rows: 0:50 taus 0-4 | 64:94 taus 5-7 | 96:126 head halo (prev 5-7)
  x5/x6:   0:60 taus 0-5 | 64:84 taus 6,7 | 84 ones (bias row) |
           96:116 head halo (prev 6,7); tail (next 0,1) via K=20 matmul
Key structure vs v2:
  - softmax 1/s via nc.vector.reciprocal_approx_fast (single custom DVE
    op) instead of InstReciprocal, whose iterative-divide ucode ran at
    ~4 cycles/elem on HW and dominated the old kernel's runtime;
  - s is replicated to the q rows by ones-weights in a second accumulated
    matmul pair, so VQ costs one reciprocal + one tensor_tensor per block;
  - cr halves share one [128,1024] psum (2 banks) -> one exp ACT per
    block (both halves share the -c2 bias);
  - conv1 bias rides an x1 ones-row, d1 bias an x5 ones-row; evictions
    split ACT (conv2, conv3, exp, d2) / DVE (conv1, d1, mult) / Pool
    (x3 halo copies) to balance engine busy times.
"""

import sys

import numpy as np

if "/opt/trn_rl_repo" not in sys.path:
    sys.path.insert(0, "/opt/trn_rl_repo")

B_FULL, T, DOUT = 1024, 512, 512
NCORES = 8
B = B_FULL // NCORES  # 128
TB = 8
NTB = T // TB  # 64
F = NTB * B  # 8192
NB = 16  # 512-col blocks
BW = F // NB  # 512

_CACHE = {}

_WSPECS = [
    ("W1T", 22, 90), ("W2M", 90, 96), ("W3C", 126, 80), ("W3T", 30, 80),
    ("CRW0", 80, 128), ("CRW1", 80, 128),
    ("QWQ0", 128, 96), ("QWQ1", 128, 96),
    ("QWS0", 128, 96), ("QWS1", 128, 96),
    ("D1W", 116, 96), ("D1T", 20, 96),
    ("D2W", 116, 80), ("D2T", 20, 80), ("FCBR", 1, 512),
]
_WOFF = {}
_WTOT = 0
for _nm, _r, _c in _WSPECS:
    _WOFF[_nm] = (_WTOT, _r, _c)
    _WTOT += _c


def _x3row(tau, c):
    return tau * 10 + c if tau < 5 else 64 + (tau - 5) * 10 + c


def _x5row(tau, c):
    return tau * 10 + c if tau < 6 else 64 + (tau - 6) * 10 + c


def _host_prep(x, w1, b1, w2, b2, w3, b3, code, d1w, d1b, d2w, d2b, fcw, fcb):
    f16 = np.float16
    P = {}

    # conv1: x2 taus -5..12 (rows (tau+5)*5+ci), x1 taus -6..14 (rows u),
    # row 21 = ones (carries b1).
    W1T = np.zeros((22, 90), np.float32)
    w1 = np.asarray(w1, np.float32)
    for t in range(-5, 13):
        for j in range(3):
            u = t + j + 5
            for c in range(5):
                W1T[u, (t + 5) * 5 + c] = w1[c, 0, j]
    W1T[21, :] = np.tile(np.asarray(b1, np.float32), 18)
    P["W1T"] = W1T.astype(f16)

    # conv2: K=90 over x2 -> x3 gapped rows
    w2 = np.asarray(w2, np.float32)
    W2M = np.zeros((90, 96), np.float32)
    for tau in range(8):
        for co in range(10):
            for j in range(5):
                t = tau + j - 2
                for ci in range(5):
                    W2M[(t + 5) * 5 + ci, _x3row(tau, co)] = w2[co, ci, j]
    P["W2M"] = W2M.astype(f16)
    P["BC2"] = np.zeros((96, 1), np.float32)
    for tau in range(8):
        for co in range(10):
            P["BC2"][_x3row(tau, co), 0] = np.asarray(b2, np.float32)[co]

    # conv3: main over x3[0:126] (incl head halo rows 96:126 = prev taus
    # 5,6,7 -> eff -3..-1), tail over x3t[0:30] (next 0,1,2 -> eff 8..10)
    w3 = np.asarray(w3, np.float32)
    W3C = np.zeros((126, 80), np.float32)
    W3T = np.zeros((30, 80), np.float32)

    def w3fill(W, r, te, ci):
        for tp in range(8):
            j = te - tp + 3
            if 0 <= j < 7:
                for o in range(10):
                    W[r, tp * 10 + o] = w3[o, ci, j]

    for tau in range(8):
        for ci in range(10):
            w3fill(W3C, _x3row(tau, ci), tau, ci)
    for i, te in enumerate((-3, -2, -1)):
        for ci in range(10):
            w3fill(W3C, 96 + i * 10 + ci, te, ci)
    for i, te in enumerate((8, 9, 10)):
        for ci in range(10):
            w3fill(W3T, i * 10 + ci, te, ci)
    P["W3C"], P["W3T"] = W3C.astype(f16), W3T.astype(f16)
    P["BC3"] = np.tile(np.asarray(b3, np.float32), 8).reshape(80, 1)

    # VQ: cr = code.T @ x4 per tau; CRW_h [80, 128] block diag
    code = np.asarray(code, np.float32)
    c2 = (code * code).sum(0)
    for h in range(2):
        CRW = np.zeros((80, 128), np.float32)
        for tl in range(4):
            tau = 4 * h + tl
            CRW[tau * 10: tau * 10 + 10, tl * 32: (tl + 1) * 32] = code
        P[f"CRW{h}"] = CRW.astype(f16)
    P["C2N"] = np.tile(-c2, 4).reshape(128, 1).astype(np.float32)

    # q = sum_h QWQ_h.T @ e_h [84 gapped rows]; s likewise with ones.
    # QWS0 gap cols = 1 so the s gap rows stay positive (recip safety).
    for h in range(2):
        QWQ = np.zeros((128, 96), np.float32)
        QWS = np.zeros((128, 96), np.float32)
        for tl in range(4):
            tau = 4 * h + tl
            for c in range(10):
                QWQ[tl * 32: (tl + 1) * 32, _x5row(tau, c)] = code[c, :]
                QWS[tl * 32: (tl + 1) * 32, _x5row(tau, c)] = 1.0
        if h == 0:
            QWS[:, 60:64] = 1.0
            QWS[:, 84:96] = 1.0
            QWQ[:, 84] = 1.0  # x5 row 84 = pqs/pqs = 1.0 (d1 bias row)
        P[f"QWQ{h}"] = QWQ.astype(f16)
        P[f"QWS{h}"] = QWS.astype(f16)

    # d1: main K=116 over x5[0:116] (data + ones row 84 + head halo
    # 96:116 = prev taus 6,7 -> eff -2,-1), tail K=20 (next 0,1 -> 8,9)
    d1w = np.asarray(d1w, np.float32)
    D1W = np.zeros((116, 96), np.float32)
    D1T = np.zeros((20, 96), np.float32)

    def dfill(W, r, te, ci, dw):
        for tp in range(8):
            j = te - tp + 2
            if 0 <= j < 5:
                for co in range(10):
                    W[r, _x5row(tp, co)] += dw[co, ci, j]

    for tau in range(8):
        for ci in range(10):
            dfill(D1W, _x5row(tau, ci), tau, ci, d1w)
    for i, te in enumerate((-2, -1)):
        for ci in range(10):
            dfill(D1W, 96 + i * 10 + ci, te, ci, d1w)
    for tp in range(8):
        for co in range(10):
            D1W[84, _x5row(tp, co)] = np.asarray(d1b, np.float32)[co]
    D1W[84, 84] = 1.0  # propagate the ones row into x6 row 84
    for i, te in enumerate((8, 9)):
        for ci in range(10):
            dfill(D1T, i * 10 + ci, te, ci, d1w)
    P["D1W"], P["D1T"] = D1W.astype(f16), D1T.astype(f16)

    # d2: same structure over x6, compact output rows tp*10+co
    d2w = np.asarray(d2w, np.float32)
    D2W = np.zeros((116, 80), np.float32)
    D2T = np.zeros((20, 80), np.float32)

    def d2fill(W, r, te, ci):
        for tp in range(8):
            j = te - tp + 2
            if 0 <= j < 5:
                for co in range(10):
                    W[r, tp * 10 + co] += d2w[co, ci, j]

    for tau in range(8):
        for ci in range(10):
            d2fill(D2W, _x5row(tau, ci), tau, ci)
    for i, te in enumerate((-2, -1)):
        for ci in range(10):
            d2fill(D2W, 96 + i * 10 + ci, te, ci)
    for i, te in enumerate((8, 9)):
        for ci in range(10):
            d2fill(D2T, i * 10 + ci, te, ci)
    P["D2W"], P["D2T"] = D2W.astype(f16), D2T.astype(f16)
    P["BD2"] = np.tile(np.asarray(d2b, np.float32), 8).reshape(80, 1)

    # fc blocks: FCB [16, 80, 2048] fp16 (4 t-blocks of 512 each)
    fcw = np.asarray(fcw, np.float32)
    FCB = np.zeros((16, 80, 2048), np.float32)
    for tb in range(NTB):
        j, k = tb // 4, tb % 4
        for tau in range(8):
            for c in range(10):
                FCB[j, tau * 10 + c, k * 512: (k + 1) * 512] = fcw[
                    :, c * 512 + tb * 8 + tau]
    P["FCB"] = FCB.astype(f16)
    P["FCBR"] = np.asarray(fcb, np.float32).reshape(1, DOUT).astype(f16)

    WALL = np.zeros((128, _WTOT), np.float16)
    for nm, (off, r, c) in _WOFF.items():
        WALL[0:r, off: off + c] = P.pop(nm)
    P["WALL"] = WALL
    CB = np.zeros((128, 4), np.float32)
    CB[0:96, 0] = P.pop("BC2")[:, 0]
    CB[0:80, 1] = P.pop("BC3")[:, 0]
    CB[0:128, 2] = P.pop("C2N")[:, 0]
    CB[0:80, 3] = P.pop("BD2")[:, 0]
    P["CB"] = CB

    # per-core conv1 inputs: X22 [22, F] (row 21 = ones) -> x1h [86, 4096]
    x = np.asarray(x, np.float32)
    xs = x.reshape(NCORES, B, T)
    xp = np.zeros((NCORES, B, T + 21), np.float32)
    xp[:, :, 6: T + 6] = xs
    tt = np.arange(NTB)[:, None] * TB + np.arange(21)[None, :]
    g = xp[:, :, tt]  # [NCORES, B, NTB, 21]
    X21 = np.ascontiguousarray(g.transpose(0, 3, 2, 1).reshape(NCORES, 21, F))
    x1h = np.zeros((NCORES, 86, 4096), np.float32)
    for q in range(4):
        r0 = 64 * (q % 2)
        c0 = 2048 * (q // 2)
        x1h[:, r0: r0 + 21, c0: c0 + 2048] = X21[:, :, 2048 * q: 2048 * (q + 1)]
        x1h[:, r0 + 21, c0: c0 + 2048] = 1.0
    P["x1_shards"] = x1h.astype(f16)
    return P


# ------------------------------------------------------------- device program
def _build_nc(debug=False, reps=1, trunc=9):
    import concourse.bacc as bacc
    import concourse.mybir as mybir
    import concourse.tile as tile
    from contextlib import ExitStack

    dt = mybir.dt
    f32 = dt.float32
    f16 = dt.float16
    AF = mybir.ActivationFunctionType
    ALU = mybir.AluOpType

    nc = bacc.Bacc()

    def din(name, shape, dt_=f16):
        return nc.declare_dram_parameter(name, list(shape), dt_, isOutput=False)

    x1_d = din("x1", (86, 4096))
    WALL_d = din("WALL", (128, _WTOT))
    CB_d = din("CB", (128, 4), f32)
    FCB_d = din("FCB", (16, 80, 2048))
    out_d = nc.declare_dram_parameter("out", [B, DOUT], f16, isOutput=True)
    dbg = {}
    if debug:
        for nm, p_ in [("dx2", 90), ("dx3", 128), ("dx4", 80), ("dx5", 128),
                       ("dx6", 128), ("dx7", 80)]:
            dbg[nm] = nc.declare_dram_parameter(nm, [p_, F], f32, isOutput=True)

    with tile.TileContext(nc) as tc, ExitStack() as ctx:
        wp = ctx.enter_context(tc.tile_pool(name="wts", bufs=1))
        ap_ = ctx.enter_context(tc.tile_pool(name="acts", bufs=1))
        pp = ctx.enter_context(tc.tile_pool(name="ps", bufs=5, space="PSUM"))
        ppcr = ctx.enter_context(tc.tile_pool(name="pcr", bufs=1, space="PSUM"))
        fcpp = ctx.enter_context(tc.tile_pool(name="fcps", bufs=1, space="PSUM"))
        fwp = ctx.enter_context(tc.tile_pool(name="fcw", bufs=8))
        stp = ctx.enter_context(tc.tile_pool(name="stp", bufs=4))
        sp = ctx.enter_context(tc.tile_pool(name="svals", bufs=1))

        x1q = []
        for q in range(4):
            t_ = ap_.tile([22, 2048], f16, tag=f"x1q{q}", name=f"x1q{q}")
            r0 = 64 * (q % 2)
            c0 = 2048 * (q // 2)
            nc.scalar.dma_start(
                out=t_[:, :], in_=x1_d[r0: r0 + 22, c0: c0 + 2048])
            x1q.append(t_)
        WALL = wp.tile([128, _WTOT], f16, tag="WALL")
        nc.sync.dma_start(out=WALL[:, :], in_=WALL_d[:, :])
        CB = wp.tile([128, 4], f32, tag="CB")
        nc.sync.dma_start(out=CB[:, :], in_=CB_d[:, :])

        def wv(nm):
            off, r, c = _WOFF[nm]
            return WALL[0:r, off: off + c]

        W1T = wv("W1T")
        W2M = wv("W2M")
        W3C, W3T = wv("W3C"), wv("W3T")
        CRW = (wv("CRW0"), wv("CRW1"))
        QWQ = (wv("QWQ0"), wv("QWQ1"))
        QWS = (wv("QWS0"), wv("QWS1"))
        D1W, D1T = wv("D1W"), wv("D1T")
        D2W, D2T = wv("D2W"), wv("D2T")
        FCBR = wv("FCBR")
        BC2 = CB[0:96, 0:1]
        BC3 = CB[0:80, 1:2]
        C2N = CB[0:128, 2:3]
        BD2 = CB[0:80, 3:4]

        ones = sp.tile([1, B], f16, tag="ones")
        nc.vector.memset(ones[:, :], 1.0)

        def mm(out, lhsT, rhs, start, stop=True):
            nc.tensor.matmul(out, lhsT, rhs, start=start, stop=stop)

        for _rep in range(reps):
            x2 = ap_.tile([90, F], f16, tag="x2")
            x3 = ap_.tile([128, F], f16, tag="x3")
            x3t = ap_.tile([32, F], f16, tag="x3t")
            x4 = ap_.tile([80, F], f16, tag="x4")
            em = ap_.tile([128, 2 * F], f16, tag="em")
            x5 = ap_.tile([128, F], f16, tag="x5")
            x6 = ap_.tile([128, F], f16, tag="x6")
            x7 = ap_.tile([80, F], f16, tag="x7")
            fws = [fwp.tile([80, 2048], f16, tag="fw", name=f"fw{_j}")
                   for _j in range(16)]
            fcp = fcpp.tile([B, DOUT], f32, tag="fcp")

            # head-halo boundary columns (block 0 reads t-block -1 = 0);
            # all other junk rows are covered by zero/ones weight columns
            nc.gpsimd.memset(x3[96:128, 0:B], 0.0)
            nc.gpsimd.memset(x3t[0:30, F - B: F], 0.0)
            nc.gpsimd.memset(x5[96:128, 0:B], 0.0)
            nc.gpsimd.memset(x6[96:128, 0:B], 0.0)

            def conv1(b):
                t = x1q[b // 4]
                c0 = (b % 4) * BW
                g0 = b * BW
                p1 = pp.tile([90, BW], f32, tag="ps", name="p1")
                mm(p1[:, :], W1T[:, :], t[:, c0: c0 + BW], True)
                nc.vector.tensor_relu(x2[:, g0: g0 + BW], p1[:, :])

            def conv2(b):
                g0 = b * BW
                p2 = pp.tile([96, BW], f32, tag="ps", name="p2")
                mm(p2[:, :], W2M[:, :], x2[:, g0: g0 + BW], True)
                nc.scalar.activation(
                    x3[0:96, g0: g0 + BW], p2[:, :], AF.Relu, bias=BC2)

            def x3cp(b):
                g0 = b * BW
                # head: prev block taus 5,6,7 (x3[64:94]) -> x3 rows 96:126
                lo = max(g0 - B, 0)
                off = lo - (g0 - B)
                nc.gpsimd.tensor_copy(
                    x3[96:126, g0 + off: g0 + BW],
                    x3[64:94, lo: lo + BW - off])
                # tail: next block taus 0,1,2 -> x3t rows 0:30
                w = min(g0 + B + BW, F) - (g0 + B)
                nc.gpsimd.tensor_copy(
                    x3t[0:30, g0: g0 + w], x3[0:30, g0 + B: g0 + B + w])

            def conv3(b):
                g0 = b * BW
                p3 = pp.tile([80, BW], f32, tag="ps", name="p3")
                mm(p3[:, :], W3C[:, :], x3[0:126, g0: g0 + BW], True,
                   stop=False)
                mm(p3[:, :], W3T[:, :], x3t[0:30, g0: g0 + BW], False)
                nc.scalar.activation(
                    x4[:, g0: g0 + BW], p3[:, :], AF.Tanh, bias=BC3)

            def vq_cr(b):
                g0 = b * BW
                pcr = ppcr.tile([128, 2 * BW], f32, tag="pcr", name="pcr")
                # the two halves land in different banks; each must be its
                # own start=True group (start clears has_written bank-wide,
                # and a start=False write would accumulate onto the
                # previous block's stale contents)
                mm(pcr[:, 0:BW], CRW[0][:, :], x4[:, g0: g0 + BW], True)
                mm(pcr[:, BW: 2 * BW], CRW[1][:, :], x4[:, g0: g0 + BW],
                   True)
                nc.scalar.activation(
                    em[:, 2 * g0: 2 * g0 + 2 * BW], pcr[:, :], AF.Exp,
                    bias=C2N, scale=2.0)

            def vq_q(b):
                g0 = b * BW
                e0 = em[:, 2 * g0: 2 * g0 + BW]
                e1 = em[:, 2 * g0 + BW: 2 * g0 + 2 * BW]
                pqq = pp.tile([96, BW], f32, tag="ps", name="pqq")
                mm(pqq[:, :], QWQ[0][:, :], e0, True, stop=False)
                mm(pqq[:, :], QWQ[1][:, :], e1, False)
                pqs = pp.tile([96, BW], f32, tag="ps", name="pqs")
                mm(pqs[:, :], QWS[0][:, :], e0, True, stop=False)
                mm(pqs[:, :], QWS[1][:, :], e1, False)
                st = stp.tile([96, BW], f32, tag="st", name="st")
                nc.vector.reciprocal_approx_fast(out=st[:, :], in_=pqs[:, :])
                with nc.allow_low_precision(reason="softmax 1/s in fp16"):
                    nc.vector.tensor_tensor(
                        x5[0:96, g0: g0 + BW], pqq[:, :], st[:, :],
                        ALU.mult)

            def x5cp(b):
                g0 = b * BW
                lo = max(g0 - B, 0)
                off = lo - (g0 - B)
                nc.vector.tensor_copy(
                    x5[96:116, g0 + off: g0 + BW],
                    x5[64:84, lo: lo + BW - off])

            def d1(b):
                g0 = b * BW
                pd1 = pp.tile([96, BW], f32, tag="ps", name="pd1")
                mm(pd1[:, :], D1W[:, :], x5[0:116, g0: g0 + BW], True,
                   stop=False)
                w = min(g0 + B + BW, F) - (g0 + B)
                mm(pd1[:, 0:w], D1T[:, :], x5[0:20, g0 + B: g0 + B + w],
                   False)
                nc.vector.tensor_relu(x6[0:96, g0: g0 + BW], pd1[:, :])

            def x6cp(b):
                g0 = b * BW
                lo = max(g0 - B, 0)
                off = lo - (g0 - B)
                nc.vector.tensor_copy(
                    x6[96:116, g0 + off: g0 + BW],
                    x6[64:84, lo: lo + BW - off])

            def d2(b):
                g0 = b * BW
                pd2 = pp.tile([80, BW], f32, tag="ps", name="pd2")
                mm(pd2[:, :], D2W[:, :], x6[0:116, g0: g0 + BW], True,
                   stop=False)
                w = min(g0 + B + BW, F) - (g0 + B)
                mm(pd2[:, 0:w], D2T[:, :], x6[0:20, g0 + B: g0 + B + w],
                   False)
                nc.scalar.activation(
                    x7[:, g0: g0 + BW], pd2[:, :], AF.Relu, bias=BD2)

            def fc(q):
                if q == 0:
                    mm(fcp[:, :], ones[0:1, 0:B], FCBR[0:1, :], True,
                       stop=False)
                for k in range(4):
                    tb = 4 * q + k
                    mm(fcp[:, :], x7[:, tb * B: (tb + 1) * B],
                       fws[q][:, k * 512: (k + 1) * 512],
                       False, stop=(tb == NTB - 1))

            # fused software pipeline; lags satisfy cross-block halo deps
            # (b+1) and the psum pool rotation (bufs=5, 6 allocs/iter)
            for i in range(NB + 10):
                if i < NB:
                    conv1(i)
                if i < 16 and trunc >= 9:
                    nc.sync.dma_start(out=fws[i][:, :], in_=FCB_d[i, :, :])
                if 0 <= i - 1 < NB and trunc >= 2:
                    conv2(i - 1)
                if 0 <= i - 2 < NB and trunc >= 3:
                    x3cp(i - 2)
                if 0 <= i - 3 < NB and trunc >= 3:
                    conv3(i - 3)
                if 0 <= i - 4 < NB and trunc >= 4:
                    vq_cr(i - 4)
                if 0 <= i - 5 < NB and trunc >= 5:
                    vq_q(i - 5)
                if 0 <= i - 6 < NB and trunc >= 6:
                    x5cp(i - 6)
                if 0 <= i - 7 < NB and trunc >= 7:
                    d1(i - 7)
                if 0 <= i - 8 < NB and trunc >= 8:
                    x6cp(i - 8)
                if 0 <= i - 9 < NB and trunc >= 8:
                    d2(i - 9)
                if 0 <= i - 10 < NB and trunc >= 9:
                    fc(i - 10)

            if debug:
                for nm, t in [("dx2", x2), ("dx3", x3), ("dx4", x4),
                              ("dx5", x5), ("dx6", x6), ("dx7", x7)]:
                    nc.gpsimd.dma_start(out=dbg[nm][:, :], in_=t[:, 0:F])

            out_sb = sp.tile([B, DOUT], f16, tag="out")
            if trunc >= 9:
                nc.scalar.activation(out_sb[:, :], fcp[:, :], AF.Tanh)
            else:
                tdump = {1: x2, 2: x3, 3: x4, 4: em, 5: x5, 6: x5,
                         7: x6, 8: x7}[trunc]
                nc.scalar.activation(out_sb[0:64, :],
                                     tdump[0:64, 0:512], AF.Copy)
            nc.sync.dma_start(out=out_d[:, :], in_=out_sb[:, :])

    nc.compile()
    return nc


def _get_nc():
    if "nc" not in _CACHE:
        _CACHE["nc"] = _build_nc()
    return _CACHE["nc"]


_COMMON = ("WALL", "CB", "FCB")


def kernel(**inputs):
    P = _host_prep(**inputs)
    nc = _get_nc()
    common = {k: P[k] for k in _COMMON}
    in_maps = [dict(common, x1=P["x1_shards"][i]) for i in range(NCORES)]
    from concourse.bass_utils import run_bass_kernel_spmd

    res = run_bass_kernel_spmd(nc, in_maps, list(range(NCORES)))
    return np.concatenate([res.results[i]["out"] for i in range(NCORES)],
                          axis=0).astype(np.float32)


if __name__ == "__main__":
    import reference

    inputs = {k: np.asarray(v) for k, v in reference.setup_inputs().items()}
    out = kernel(**inputs)
    exp = np.asarray(reference.reference(**inputs))
    err = np.abs(out - exp).max() / (np.abs(exp).max() + 1e-30)
    print("Relative error:", err)


# revision 3
# speedup vs baseline: 1.0303x; 1.0303x over previous
"""VQ-codebook autoencoder Trainium2 kernel (v3).

Data-parallel over 8 NeuronCores: batch 1024 -> 8 x 128.

Per-core layout: feature rows on partitions, (t-block, batch) on the free
dim: F = NTB * B = 64 * 128 = 8192, free index = tb*128 + b, TB=8 taus per
t-block.  fp16 activations, fp32 psum.

Every engine access needs a 32-aligned partition base, so tau-row maps are
gapped to put halo sources/destinations at 32-aligned rows; the gaps live
inside matmul outputs as zero weight columns (no extra instruction cost):
  x3 rows: 0:50 taus 0-4 | 64:94 taus 5-7 | 96:126 head halo (prev 5-7)
  x5/x6:   0:60 taus 0-5 | 64:84 taus 6,7 | 84 ones (bias row) |
           96:116 head halo (prev 6,7); tail (next 0,1) via K=20 matmul
Key structure vs v2:
  - softmax 1/s via nc.vector.reciprocal_approx_fast (single custom DVE
    op) instead of InstReciprocal, whose iterative-divide ucode ran at
    ~4 cycles/elem on HW and dominated the old kernel's runtime;
  - s is replicated to the q rows by ones-weights in a second accumulated
    matmul pair, so VQ costs one reciprocal + one tensor_tensor per block;
  - cr halves share one [128,1024] psum (2 banks) -> one exp ACT per
    block (both halves share the -c2 bias);
  - conv1 bias rides an x1 ones-row, d1 bias an x5 ones-row; evictions
    split ACT (conv2, conv3, exp, d2) / DVE (conv1, d1, mult) / Pool
    (x3 halo copies) to balance engine busy times.
"""

import sys

import numpy as np

if "/opt/trn_rl_repo" not in sys.path:
    sys.path.insert(0, "/opt/trn_rl_repo")

B_FULL, T, DOUT = 1024, 512, 512
NCORES = 8
B = B_FULL // NCORES  # 128
TB = 8
NTB = T // TB  # 64
F = NTB * B  # 8192
NB = 16  # 512-col blocks
BW = F // NB  # 512

_CACHE = {}

_WSPECS = [
    ("W1T", 22, 90), ("W2M", 90, 96), ("W3C", 126, 80), ("W3T", 30, 80),
    ("CRW0", 80, 128), ("CRW1", 80, 128),
    ("QWQ0", 128, 96), ("QWQ1", 128, 96),
    ("QWS0", 128, 96), ("QWS1", 128, 96),
    ("D1W", 116, 96), ("D1T", 20, 96),
    ("D2W", 116, 80), ("D2T", 20, 80), ("FCBR", 1, 512),
]
_WOFF = {}
_WTOT = 0
for _nm, _r, _c in _WSPECS:
    _WOFF[_nm] = (_WTOT, _r, _c)
    _WTOT += _c


def _x3row(tau, c):
    return tau * 10 + c if tau < 5 else 64 + (tau - 5) * 10 + c


def _x5row(tau, c):
    return tau * 10 + c if tau < 6 else 64 + (tau - 6) * 10 + c


def _host_prep(x, w1, b1, w2, b2, w3, b3, code, d1w, d1b, d2w, d2b, fcw, fcb):
    f16 = np.float16
    P = {}

    # conv1: x2 taus -5..12 (rows (tau+5)*5+ci), x1 taus -6..14 (rows u),
    # row 21 = ones (carries b1).
    W1T = np.zeros((22, 90), np.float32)
    w1 = np.asarray(w1, np.float32)
    for t in range(-5, 13):
        for j in range(3):
            u = t + j + 5
            for c in range(5):
                W1T[u, (t + 5) * 5 + c] = w1[c, 0, j]
    W1T[21, :] = np.tile(np.asarray(b1, np.float32), 18)
    P["W1T"] = W1T.astype(f16)

    # conv2: K=90 over x2 -> x3 gapped rows
    w2 = np.asarray(w2, np.float32)
    W2M = np.zeros((90, 96), np.float32)
    for tau in range(8):
        for co in range(10):
            for j in range(5):
                t = tau + j - 2
                for ci in range(5):
                    W2M[(t + 5) * 5 + ci, _x3row(tau, co)] = w2[co, ci, j]
    P["W2M"] = W2M.astype(f16)
    P["BC2"] = np.zeros((96, 1), np.float32)
    for tau in range(8):
        for co in range(10):
            P["BC2"][_x3row(tau, co), 0] = np.asarray(b2, np.float32)[co]

    # conv3: main over x3[0:126] (incl head halo rows 96:126 = prev taus
    # 5,6,7 -> eff -3..-1), tail over x3t[0:30] (next 0,1,2 -> eff 8..10)
    w3 = np.asarray(w3, np.float32)
    W3C = np.zeros((126, 80), np.float32)
    W3T = np.zeros((30, 80), np.float32)

    def w3fill(W, r, te, ci):
        for tp in range(8):
            j = te - tp + 3
            if 0 <= j < 7:
                for o in range(10):
                    W[r, tp * 10 + o] = w3[o, ci, j]

    for tau in range(8):
        for ci in range(10):
            w3fill(W3C, _x3row(tau, ci), tau, ci)
    for i, te in enumerate((-3, -2, -1)):
        for ci in range(10):
            w3fill(W3C, 96 + i * 10 + ci, te, ci)
    for i, te in enumerate((8, 9, 10)):
        for ci in range(10):
            w3fill(W3T, i * 10 + ci, te, ci)
    P["W3C"], P["W3T"] = W3C.astype(f16), W3T.astype(f16)
    P["BC3"] = np.tile(np.asarray(b3, np.float32), 8).reshape(80, 1)

    # VQ: cr = code.T @ x4 per tau; CRW_h [80, 128] block diag
    code = np.asarray(code, np.float32)
    c2 = (code * code).sum(0)
    for h in range(2):
        CRW = np.zeros((80, 128), np.float32)
        for tl in range(4):
            tau = 4 * h + tl
            CRW[tau * 10: tau * 10 + 10, tl * 32: (tl + 1) * 32] = code
        P[f"CRW{h}"] = CRW.astype(f16)
    P["C2N"] = np.tile(-c2, 4).reshape(128, 1).astype(np.float32)

    # q = sum_h QWQ_h.T @ e_h [84 gapped rows]; s likewise with ones.
    # QWS0 gap cols = 1 so the s gap rows stay positive (recip safety).
    for h in range(2):
        QWQ = np.zeros((128, 96), np.float32)
        QWS = np.zeros((128, 96), np.float32)
        for tl in range(4):
            tau = 4 * h + tl
            for c in range(10):
                QWQ[tl * 32: (tl + 1) * 32, _x5row(tau, c)] = code[c, :]
                QWS[tl * 32: (tl + 1) * 32, _x5row(tau, c)] = 1.0
        if h == 0:
            QWS[:, 60:64] = 1.0
            QWS[:, 84:96] = 1.0
            QWQ[:, 84] = 1.0  # x5 row 84 = pqs/pqs = 1.0 (d1 bias row)
        P[f"QWQ{h}"] = QWQ.astype(f16)
        P[f"QWS{h}"] = QWS.astype(f16)

    # d1: main K=116 over x5[0:116] (data + ones row 84 + head halo
    # 96:116 = prev taus 6,7 -> eff -2,-1), tail K=20 (next 0,1 -> 8,9)
    d1w = np.asarray(d1w, np.float32)
    D1W = np.zeros((116, 96), np.float32)
    D1T = np.zeros((20, 96), np.float32)

    def dfill(W, r, te, ci, dw):
        for tp in range(8):
            j = te - tp + 2
            if 0 <= j < 5:
                for co in range(10):
                    W[r, _x5row(tp, co)] += dw[co, ci, j]

    for tau in range(8):
        for ci in range(10):
            dfill(D1W, _x5row(tau, ci), tau, ci, d1w)
    for i, te in enumerate((-2, -1)):
        for ci in range(10):
            dfill(D1W, 96 + i * 10 + ci, te, ci, d1w)
    for tp in range(8):
        for co in range(10):
            D1W[84, _x5row(tp, co)] = np.asarray(d1b, np.float32)[co]
    D1W[84, 84] = 1.0  # propagate the ones row into x6 row 84
    for i, te in enumerate((8, 9)):
        for ci in range(10):
            dfill(D1T, i * 10 + ci, te, ci, d1w)
    P["D1W"], P["D1T"] = D1W.astype(f16), D1T.astype(f16)

    # d2: same structure over x6, compact output rows tp*10+co
    d2w = np.asarray(d2w, np.float32)
    D2W = np.zeros((116, 80), np.float32)
    D2T = np.zeros((20, 80), np.float32)

    def d2fill(W, r, te, ci):
        for tp in range(8):
            j = te - tp + 2
            if 0 <= j < 5:
                for co in range(10):
                    W[r, tp * 10 + co] += d2w[co, ci, j]

    for tau in range(8):
        for ci in range(10):
            d2fill(D2W, _x5row(tau, ci), tau, ci)
    for i, te in enumerate((-2, -1)):
        for ci in range(10):
            d2fill(D2W, 96 + i * 10 + ci, te, ci)
    for i, te in enumerate((8, 9)):
        for ci in range(10):
            d2fill(D2T, i * 10 + ci, te, ci)
    P["D2W"], P["D2T"] = D2W.astype(f16), D2T.astype(f16)
    P["BD2"] = np.tile(np.asarray(d2b, np.float32), 8).reshape(80, 1)

    # fc blocks: FCB [16, 80, 2048] fp16 (4 t-blocks of 512 each)
    fcw = np.asarray(fcw, np.float32)
    FCB = np.zeros((16, 80, 2048), np.float32)
    for tb in range(NTB):
        j, k = tb // 4, tb % 4
        for tau in range(8):
            for c in range(10):
                FCB[j, tau * 10 + c, k * 512: (k + 1) * 512] = fcw[
                    :, c * 512 + tb * 8 + tau]
    P["FCB"] = FCB.astype(f16)
    P["FCBR"] = np.asarray(fcb, np.float32).reshape(1, DOUT).astype(f16)

    WALL = np.zeros((128, _WTOT), np.float16)
    for nm, (off, r, c) in _WOFF.items():
        WALL[0:r, off: off + c] = P.pop(nm)
    P["WALL"] = WALL
    CB = np.zeros((128, 4), np.float32)
    CB[0:96, 0] = P.pop("BC2")[:, 0]
    CB[0:80, 1] = P.pop("BC3")[:, 0]
    CB[0:128, 2] = P.pop("C2N")[:, 0]
    CB[0:80, 3] = P.pop("BD2")[:, 0]
    P["CB"] = CB

    # per-core conv1 inputs: X22 [22, F] (row 21 = ones) -> x1h [86, 4096]
    x = np.asarray(x, np.float32)
    xs = x.reshape(NCORES, B, T)
    xp = np.zeros((NCORES, B, T + 21), np.float32)
    xp[:, :, 6: T + 6] = xs
    tt = np.arange(NTB)[:, None] * TB + np.arange(21)[None, :]
    g = xp[:, :, tt]  # [NCORES, B, NTB, 21]
    X21 = np.ascontiguousarray(g.transpose(0, 3, 2, 1).reshape(NCORES, 21, F))
    x1h = np.zeros((NCORES, 86, 4096), np.float32)
    for q in range(4):
        r0 = 64 * (q % 2)
        c0 = 2048 * (q // 2)
        x1h[:, r0: r0 + 21, c0: c0 + 2048] = X21[:, :, 2048 * q: 2048 * (q + 1)]
        x1h[:, r0 + 21, c0: c0 + 2048] = 1.0
    P["x1_shards"] = x1h.astype(f16)
    return P


# ------------------------------------------------------------- device program
def _build_nc(debug=False, reps=1, trunc=9):
    import concourse.bacc as bacc
    import concourse.mybir as mybir
    import concourse.tile as tile
    from contextlib import ExitStack

    dt = mybir.dt
    f32 = dt.float32
    f16 = dt.float16
    AF = mybir.ActivationFunctionType
    ALU = mybir.AluOpType

    nc = bacc.Bacc()

    def din(name, shape, dt_=f16):
        return nc.declare_dram_parameter(name, list(shape), dt_, isOutput=False)

    x1_d = din("x1", (86, 4096))
    WALL_d = din("WALL", (128, _WTOT))
    CB_d = din("CB", (128, 4), f32)
    FCB_d = din("FCB", (16, 80, 2048))
    out_d = nc.declare_dram_parameter("out", [B, DOUT], f16, isOutput=True)
    dbg = {}
    if debug:
        for nm, p_ in [("dx2", 90), ("dx3", 128), ("dx4", 80), ("dx5", 128),
                       ("dx6", 128), ("dx7", 80)]:
            dbg[nm] = nc.declare_dram_parameter(nm, [p_, F], f32, isOutput=True)

    with tile.TileContext(nc) as tc, ExitStack() as ctx:
        wp = ctx.enter_context(tc.tile_pool(name="wts", bufs=1))
        ap_ = ctx.enter_context(tc.tile_pool(name="acts", bufs=1))
        pp = ctx.enter_context(tc.tile_pool(name="ps", bufs=5, space="PSUM"))
        ppcr = ctx.enter_context(tc.tile_pool(name="pcr", bufs=1, space="PSUM"))
        fcpp = ctx.enter_context(tc.tile_pool(name="fcps", bufs=1, space="PSUM"))
        fwp = ctx.enter_context(tc.tile_pool(name="fcw", bufs=8))
        stp = ctx.enter_context(tc.tile_pool(name="stp", bufs=4))
        sp = ctx.enter_context(tc.tile_pool(name="svals", bufs=1))

        WALL = wp.tile([128, _WTOT], f16, tag="WALL")
        nc.sync.dma_start(out=WALL[:, 0:90], in_=WALL_d[:, 0:90])
        x1q = []
        for q in range(4):
            t_ = ap_.tile([22, 2048], f16, tag=f"x1q{q}", name=f"x1q{q}")
            r0 = 64 * (q % 2)
            c0 = 2048 * (q // 2)
            if q == 0:
                nc.scalar.dma_start(
                    out=t_[:, 0:512], in_=x1_d[r0: r0 + 22, c0: c0 + 512])
                nc.scalar.dma_start(
                    out=t_[:, 512:2048],
                    in_=x1_d[r0: r0 + 22, c0 + 512: c0 + 2048])
            else:
                nc.scalar.dma_start(
                    out=t_[:, :], in_=x1_d[r0: r0 + 22, c0: c0 + 2048])
            x1q.append(t_)
        nc.sync.dma_start(out=WALL[:, 90:_WTOT], in_=WALL_d[:, 90:_WTOT])
        CB = wp.tile([128, 4], f32, tag="CB")
        nc.sync.dma_start(out=CB[:, :], in_=CB_d[:, :])

        def wv(nm):
            off, r, c = _WOFF[nm]
            return WALL[0:r, off: off + c]

        W1T = wv("W1T")
        W2M = wv("W2M")
        W3C, W3T = wv("W3C"), wv("W3T")
        CRW = (wv("CRW0"), wv("CRW1"))
        QWQ = (wv("QWQ0"), wv("QWQ1"))
        QWS = (wv("QWS0"), wv("QWS1"))
        D1W, D1T = wv("D1W"), wv("D1T")
        D2W, D2T = wv("D2W"), wv("D2T")
        FCBR = wv("FCBR")
        BC2 = CB[0:96, 0:1]
        BC3 = CB[0:80, 1:2]
        C2N = CB[0:128, 2:3]
        BD2 = CB[0:80, 3:4]

        ones = sp.tile([1, B], f16, tag="ones")
        nc.vector.memset(ones[:, :], 1.0)

        def mm(out, lhsT, rhs, start, stop=True):
            nc.tensor.matmul(out, lhsT, rhs, start=start, stop=stop)

        for _rep in range(reps):
            x2 = ap_.tile([90, F], f16, tag="x2")
            x3 = ap_.tile([128, F], f16, tag="x3")
            x3t = ap_.tile([32, F], f16, tag="x3t")
            x4 = ap_.tile([80, F], f16, tag="x4")
            em = ap_.tile([128, 2 * F], f16, tag="em")
            x5 = ap_.tile([128, F], f16, tag="x5")
            x6 = ap_.tile([128, F], f16, tag="x6")
            x7 = ap_.tile([80, F], f16, tag="x7")
            fws = [fwp.tile([80, 2048], f16, tag="fw", name=f"fw{_j}")
                   for _j in range(16)]
            fcp = fcpp.tile([B, DOUT], f32, tag="fcp")

            # head-halo boundary columns (block 0 reads t-block -1 = 0);
            # all other junk rows are covered by zero/ones weight columns
            nc.gpsimd.memset(x3[96:128, 0:B], 0.0)
            nc.gpsimd.memset(x3t[0:30, F - B: F], 0.0)
            nc.gpsimd.memset(x5[96:128, 0:B], 0.0)
            nc.gpsimd.memset(x6[96:128, 0:B], 0.0)

            def conv1(b):
                t = x1q[b // 4]
                c0 = (b % 4) * BW
                g0 = b * BW
                p1 = pp.tile([90, BW], f32, tag="ps", name="p1")
                mm(p1[:, :], W1T[:, :], t[:, c0: c0 + BW], True)
                nc.vector.tensor_relu(x2[:, g0: g0 + BW], p1[:, :])

            def conv2(b):
                g0 = b * BW
                p2 = pp.tile([96, BW], f32, tag="ps", name="p2")
                mm(p2[:, :], W2M[:, :], x2[:, g0: g0 + BW], True)
                nc.scalar.activation(
                    x3[0:96, g0: g0 + BW], p2[:, :], AF.Relu, bias=BC2)

            def x3cp(b):
                g0 = b * BW
                # head: prev block taus 5,6,7 (x3[64:94]) -> x3 rows 96:126
                lo = max(g0 - B, 0)
                off = lo - (g0 - B)
                nc.gpsimd.tensor_copy(
                    x3[96:126, g0 + off: g0 + BW],
                    x3[64:94, lo: lo + BW - off])
                # tail: next block taus 0,1,2 -> x3t rows 0:30
                w = min(g0 + B + BW, F) - (g0 + B)
                nc.gpsimd.tensor_copy(
                    x3t[0:30, g0: g0 + w], x3[0:30, g0 + B: g0 + B + w])

            def conv3(b):
                g0 = b * BW
                p3 = pp.tile([80, BW], f32, tag="ps", name="p3")
                mm(p3[:, :], W3C[:, :], x3[0:126, g0: g0 + BW], True,
                   stop=False)
                mm(p3[:, :], W3T[:, :], x3t[0:30, g0: g0 + BW], False)
                nc.scalar.activation(
                    x4[:, g0: g0 + BW], p3[:, :], AF.Tanh, bias=BC3)

            def vq_cr(b):
                g0 = b * BW
                pcr = ppcr.tile([128, 2 * BW], f32, tag="pcr", name="pcr")
                # the two halves land in different banks; each must be its
                # own start=True group (start clears has_written bank-wide,
                # and a start=False write would accumulate onto the
                # previous block's stale contents)
                mm(pcr[:, 0:BW], CRW[0][:, :], x4[:, g0: g0 + BW], True)
                mm(pcr[:, BW: 2 * BW], CRW[1][:, :], x4[:, g0: g0 + BW],
                   True)
                nc.scalar.activation(
                    em[:, 2 * g0: 2 * g0 + 2 * BW], pcr[:, :], AF.Exp,
                    bias=C2N, scale=2.0)

            def vq_q(b):
                g0 = b * BW
                e0 = em[:, 2 * g0: 2 * g0 + BW]
                e1 = em[:, 2 * g0 + BW: 2 * g0 + 2 * BW]
                pqq = pp.tile([96, BW], f32, tag="ps", name="pqq")
                mm(pqq[:, :], QWQ[0][:, :], e0, True, stop=False)
                mm(pqq[:, :], QWQ[1][:, :], e1, False)
                pqs = pp.tile([96, BW], f32, tag="ps", name="pqs")
                mm(pqs[:, :], QWS[0][:, :], e0, True, stop=False)
                mm(pqs[:, :], QWS[1][:, :], e1, False)
                st = stp.tile([96, BW], f32, tag="st", name="st")
                nc.vector.reciprocal_approx_fast(out=st[:, :], in_=pqs[:, :])
                with nc.allow_low_precision(reason="softmax 1/s in fp16"):
                    nc.vector.tensor_tensor(
                        x5[0:96, g0: g0 + BW], pqq[:, :], st[:, :],
                        ALU.mult)

            def x5cp(b):
                g0 = b * BW
                lo = max(g0 - B, 0)
                off = lo - (g0 - B)
                nc.vector.tensor_copy(
                    x5[96:116, g0 + off: g0 + BW],
                    x5[64:84, lo: lo + BW - off])

            def d1(b):
                g0 = b * BW
                pd1 = pp.tile([96, BW], f32, tag="ps", name="pd1")
                mm(pd1[:, :], D1W[:, :], x5[0:116, g0: g0 + BW], True,
                   stop=False)
                w = min(g0 + B + BW, F) - (g0 + B)
                mm(pd1[:, 0:w], D1T[:, :], x5[0:20, g0 + B: g0 + B + w],
                   False)
                nc.vector.tensor_relu(x6[0:96, g0: g0 + BW], pd1[:, :])

            def x6cp(b):
                g0 = b * BW
                lo = max(g0 - B, 0)
                off = lo - (g0 - B)
                nc.vector.tensor_copy(
                    x6[96:116, g0 + off: g0 + BW],
                    x6[64:84, lo: lo + BW - off])

            def d2(b):
                g0 = b * BW
                pd2 = pp.tile([80, BW], f32, tag="ps", name="pd2")
                mm(pd2[:, :], D2W[:, :], x6[0:116, g0: g0 + BW], True,
                   stop=False)
                w = min(g0 + B + BW, F) - (g0 + B)
                mm(pd2[:, 0:w], D2T[:, :], x6[0:20, g0 + B: g0 + B + w],
                   False)
                nc.scalar.activation(
                    x7[:, g0: g0 + BW], pd2[:, :], AF.Relu, bias=BD2)

            def fc(q):
                if q == 0:
                    mm(fcp[:, :], ones[0:1, 0:B], FCBR[0:1, :], True,
                       stop=False)
                for k in range(4):
                    tb = 4 * q + k
                    mm(fcp[:, :], x7[:, tb * B: (tb + 1) * B],
                       fws[q][:, k * 512: (k + 1) * 512],
                       False, stop=(tb == NTB - 1))

            # fused software pipeline; lags satisfy cross-block halo deps
            # (b+1) and the psum pool rotation (bufs=5, 6 allocs/iter)
            for i in range(NB + 10):
                if i < NB:
                    conv1(i)
                if i < 16 and trunc >= 9:
                    nc.sync.dma_start(out=fws[i][:, :], in_=FCB_d[i, :, :])
                if 0 <= i - 1 < NB and trunc >= 2:
                    conv2(i - 1)
                if 0 <= i - 2 < NB and trunc >= 3:
                    x3cp(i - 2)
                if 0 <= i - 3 < NB and trunc >= 3:
                    conv3(i - 3)
                if 0 <= i - 4 < NB and trunc >= 4:
                    vq_cr(i - 4)
                if 0 <= i - 5 < NB and trunc >= 5:
                    vq_q(i - 5)
                if 0 <= i - 6 < NB and trunc >= 6:
                    x5cp(i - 6)
                if 0 <= i - 7 < NB and trunc >= 7:
                    d1(i - 7)
                if 0 <= i - 8 < NB and trunc >= 8:
                    x6cp(i - 8)
                if 0 <= i - 9 < NB and trunc >= 8:
                    d2(i - 9)
                if 0 <= i - 10 < NB and trunc >= 9:
                    fc(i - 10)

            if debug:
                for nm, t in [("dx2", x2), ("dx3", x3), ("dx4", x4),
                              ("dx5", x5), ("dx6", x6), ("dx7", x7)]:
                    nc.gpsimd.dma_start(out=dbg[nm][:, :], in_=t[:, 0:F])

            out_sb = sp.tile([B, DOUT], f16, tag="out")
            if trunc >= 9:
                nc.scalar.activation(out_sb[:, :], fcp[:, :], AF.Tanh)
            else:
                tdump = {1: x2, 2: x3, 3: x4, 4: em, 5: x5, 6: x5,
                         7: x6, 8: x7}[trunc]
                nc.scalar.activation(out_sb[0:64, :],
                                     tdump[0:64, 0:512], AF.Copy)
            nc.sync.dma_start(out=out_d[:, :], in_=out_sb[:, :])

    nc.compile()
    return nc


def _get_nc():
    if "nc" not in _CACHE:
        _CACHE["nc"] = _build_nc()
    return _CACHE["nc"]


_COMMON = ("WALL", "CB", "FCB")


def kernel(**inputs):
    P = _host_prep(**inputs)
    nc = _get_nc()
    common = {k: P[k] for k in _COMMON}
    in_maps = [dict(common, x1=P["x1_shards"][i]) for i in range(NCORES)]
    from concourse.bass_utils import run_bass_kernel_spmd

    res = run_bass_kernel_spmd(nc, in_maps, list(range(NCORES)))
    return np.concatenate([res.results[i]["out"] for i in range(NCORES)],
                          axis=0).astype(np.float32)


if __name__ == "__main__":
    import reference

    inputs = {k: np.asarray(v) for k, v in reference.setup_inputs().items()}
    out = kernel(**inputs)
    exp = np.asarray(reference.reference(**inputs))
    err = np.abs(out - exp).max() / (np.abs(exp).max() + 1e-30)
    print("Relative error:", err)


# revision 4
# speedup vs baseline: 1.2128x; 1.1771x over previous
"""VQ-codebook autoencoder Trainium2 kernel (v3).

Data-parallel over 8 NeuronCores: batch 1024 -> 8 x 128.

Per-core layout: feature rows on partitions, (t-block, batch) on the free
dim: F = NTB * B = 64 * 128 = 8192, free index = tb*128 + b, TB=8 taus per
t-block.  fp16 activations, fp32 psum.

Every engine access needs a 32-aligned partition base, so tau-row maps are
gapped to put halo sources/destinations at 32-aligned rows; the gaps live
inside matmul outputs as zero weight columns (no extra instruction cost):
  x3 rows: 0:50 taus 0-4 | 64:94 taus 5-7 | 96:126 head halo (prev 5-7)
  x5/x6:   0:60 taus 0-5 | 64:84 taus 6,7 | 84 ones (bias row) |
           96:116 head halo (prev 6,7); tail (next 0,1) via K=20 matmul
Key structure vs v2:
  - softmax 1/s via nc.vector.reciprocal_approx_fast (single custom DVE
    op) instead of InstReciprocal, whose iterative-divide ucode ran at
    ~4 cycles/elem on HW and dominated the old kernel's runtime;
  - s is replicated to the q rows by ones-weights in a second accumulated
    matmul pair, so VQ costs one reciprocal + one tensor_tensor per block;
  - cr halves share one [128,1024] psum (2 banks) -> one exp ACT per
    block (both halves share the -c2 bias);
  - conv1 bias rides an x1 ones-row, d1 bias an x5 ones-row; evictions
    split ACT (conv2, conv3, exp, d2) / DVE (conv1, d1, mult) / Pool
    (x3 halo copies) to balance engine busy times.
"""

import sys

import numpy as np

if "/opt/trn_rl_repo" not in sys.path:
    sys.path.insert(0, "/opt/trn_rl_repo")

B_FULL, T, DOUT = 1024, 512, 512
NCORES = 8
B = B_FULL // NCORES  # 128
TB = 8
NTB = T // TB  # 64
F = NTB * B  # 8192
NB = 16  # 512-col blocks
BW = F // NB  # 512

_CACHE = {}

_WSPECS = [
    ("W1T", 22, 90), ("W2M", 90, 96), ("W3C", 126, 80), ("W3T", 30, 80),
    ("CRW0", 80, 128), ("CRW1", 80, 128),
    ("QWQ0", 128, 96), ("QWQ1", 128, 96),
    ("QWS0", 128, 96), ("QWS1", 128, 96),
    ("D1W", 116, 96), ("D1T", 20, 96),
    ("D2W", 116, 80), ("D2T", 20, 80), ("FCBR", 1, 512),
]
_WOFF = {}
_WTOT = 0
for _nm, _r, _c in _WSPECS:
    _WOFF[_nm] = (_WTOT, _r, _c)
    _WTOT += _c


def _x3row(tau, c):
    return tau * 10 + c if tau < 5 else 64 + (tau - 5) * 10 + c


def _x5row(tau, c):
    return tau * 10 + c if tau < 6 else 64 + (tau - 6) * 10 + c


def _host_prep(x, w1, b1, w2, b2, w3, b3, code, d1w, d1b, d2w, d2b, fcw, fcb):
    f16 = np.float16
    P = {}

    # conv1: x2 taus -5..12 (rows (tau+5)*5+ci), x1 taus -6..14 (rows u),
    # row 21 = ones (carries b1).
    W1T = np.zeros((22, 90), np.float32)
    w1 = np.asarray(w1, np.float32)
    for t in range(-5, 13):
        for j in range(3):
            u = t + j + 5
            for c in range(5):
                W1T[u, (t + 5) * 5 + c] = w1[c, 0, j]
    W1T[21, :] = np.tile(np.asarray(b1, np.float32), 18)
    P["W1T"] = W1T.astype(f16)

    # conv2: K=90 over x2 -> x3 gapped rows
    w2 = np.asarray(w2, np.float32)
    W2M = np.zeros((90, 96), np.float32)
    for tau in range(8):
        for co in range(10):
            for j in range(5):
                t = tau + j - 2
                for ci in range(5):
                    W2M[(t + 5) * 5 + ci, _x3row(tau, co)] = w2[co, ci, j]
    P["W2M"] = W2M.astype(f16)
    P["BC2"] = np.zeros((96, 1), np.float32)
    for tau in range(8):
        for co in range(10):
            P["BC2"][_x3row(tau, co), 0] = np.asarray(b2, np.float32)[co]

    # conv3: main over x3[0:126] (incl head halo rows 96:126 = prev taus
    # 5,6,7 -> eff -3..-1), tail over x3t[0:30] (next 0,1,2 -> eff 8..10)
    w3 = np.asarray(w3, np.float32)
    W3C = np.zeros((126, 80), np.float32)
    W3T = np.zeros((30, 80), np.float32)

    def w3fill(W, r, te, ci):
        for tp in range(8):
            j = te - tp + 3
            if 0 <= j < 7:
                for o in range(10):
                    W[r, tp * 10 + o] = w3[o, ci, j]

    for tau in range(8):
        for ci in range(10):
            w3fill(W3C, _x3row(tau, ci), tau, ci)
    for i, te in enumerate((-3, -2, -1)):
        for ci in range(10):
            w3fill(W3C, 96 + i * 10 + ci, te, ci)
    for i, te in enumerate((8, 9, 10)):
        for ci in range(10):
            w3fill(W3T, i * 10 + ci, te, ci)
    P["W3C"], P["W3T"] = W3C.astype(f16), W3T.astype(f16)
    P["BC3"] = np.tile(np.asarray(b3, np.float32), 8).reshape(80, 1)

    # VQ: cr = code.T @ x4 per tau; CRW_h [80, 128] block diag
    code = np.asarray(code, np.float32)
    c2 = (code * code).sum(0)
    for h in range(2):
        CRW = np.zeros((80, 128), np.float32)
        for tl in range(4):
            tau = 4 * h + tl
            CRW[tau * 10: tau * 10 + 10, tl * 32: (tl + 1) * 32] = code
        P[f"CRW{h}"] = CRW.astype(f16)
    P["C2N"] = np.tile(-c2, 4).reshape(128, 1).astype(np.float32)

    # q = sum_h QWQ_h.T @ e_h [84 gapped rows]; s likewise with ones.
    # QWS0 gap cols = 1 so the s gap rows stay positive (recip safety).
    for h in range(2):
        QWQ = np.zeros((128, 96), np.float32)
        QWS = np.zeros((128, 96), np.float32)
        for tl in range(4):
            tau = 4 * h + tl
            for c in range(10):
                QWQ[tl * 32: (tl + 1) * 32, _x5row(tau, c)] = code[c, :]
                QWS[tl * 32: (tl + 1) * 32, _x5row(tau, c)] = 1.0
        if h == 0:
            QWS[:, 60:64] = 1.0
            QWS[:, 84:96] = 1.0
            QWQ[:, 84] = 1.0  # x5 row 84 = pqs/pqs = 1.0 (d1 bias row)
        P[f"QWQ{h}"] = QWQ.astype(f16)
        P[f"QWS{h}"] = QWS.astype(f16)

    # d1: main K=116 over x5[0:116] (data + ones row 84 + head halo
    # 96:116 = prev taus 6,7 -> eff -2,-1), tail K=20 (next 0,1 -> 8,9)
    d1w = np.asarray(d1w, np.float32)
    D1W = np.zeros((116, 96), np.float32)
    D1T = np.zeros((20, 96), np.float32)

    def dfill(W, r, te, ci, dw):
        for tp in range(8):
            j = te - tp + 2
            if 0 <= j < 5:
                for co in range(10):
                    W[r, _x5row(tp, co)] += dw[co, ci, j]

    for tau in range(8):
        for ci in range(10):
            dfill(D1W, _x5row(tau, ci), tau, ci, d1w)
    for i, te in enumerate((-2, -1)):
        for ci in range(10):
            dfill(D1W, 96 + i * 10 + ci, te, ci, d1w)
    for tp in range(8):
        for co in range(10):
            D1W[84, _x5row(tp, co)] = np.asarray(d1b, np.float32)[co]
    D1W[84, 84] = 1.0  # propagate the ones row into x6 row 84
    for i, te in enumerate((8, 9)):
        for ci in range(10):
            dfill(D1T, i * 10 + ci, te, ci, d1w)
    P["D1W"], P["D1T"] = D1W.astype(f16), D1T.astype(f16)

    # d2: same structure over x6, compact output rows tp*10+co
    d2w = np.asarray(d2w, np.float32)
    D2W = np.zeros((116, 80), np.float32)
    D2T = np.zeros((20, 80), np.float32)

    def d2fill(W, r, te, ci):
        for tp in range(8):
            j = te - tp + 2
            if 0 <= j < 5:
                for co in range(10):
                    W[r, tp * 10 + co] += d2w[co, ci, j]

    for tau in range(8):
        for ci in range(10):
            d2fill(D2W, _x5row(tau, ci), tau, ci)
    for i, te in enumerate((-2, -1)):
        for ci in range(10):
            d2fill(D2W, 96 + i * 10 + ci, te, ci)
    for i, te in enumerate((8, 9)):
        for ci in range(10):
            d2fill(D2T, i * 10 + ci, te, ci)
    P["D2W"], P["D2T"] = D2W.astype(f16), D2T.astype(f16)
    P["BD2"] = np.tile(np.asarray(d2b, np.float32), 8).reshape(80, 1)

    # fc blocks: FCB [16, 80, 2048] fp16 (4 t-blocks of 512 each)
    fcw = np.asarray(fcw, np.float32)
    FCB = np.zeros((16, 80, 2048), np.float32)
    for tb in range(NTB):
        j, k = tb // 4, tb % 4
        for tau in range(8):
            for c in range(10):
                FCB[j, tau * 10 + c, k * 512: (k + 1) * 512] = fcw[
                    :, c * 512 + tb * 8 + tau]
    P["FCB"] = FCB.astype(f16)
    P["FCBR"] = np.asarray(fcb, np.float32).reshape(1, DOUT).astype(f16)

    WALL = np.zeros((128, _WTOT), np.float16)
    for nm, (off, r, c) in _WOFF.items():
        WALL[0:r, off: off + c] = P.pop(nm)
    P["WALL"] = WALL
    CB = np.zeros((128, 4), np.float32)
    CB[0:96, 0] = P.pop("BC2")[:, 0]
    CB[0:80, 1] = P.pop("BC3")[:, 0]
    CB[0:128, 2] = P.pop("C2N")[:, 0]
    CB[0:80, 3] = P.pop("BD2")[:, 0]
    P["CB"] = CB

    # per-core conv1 inputs: X22 [22, F] (row 21 = ones) -> x1h [86, 4096]
    x = np.asarray(x, np.float32)
    xs = x.reshape(NCORES, B, T)
    xp = np.zeros((NCORES, B, T + 21), np.float32)
    xp[:, :, 6: T + 6] = xs
    tt = np.arange(NTB)[:, None] * TB + np.arange(21)[None, :]
    g = xp[:, :, tt]  # [NCORES, B, NTB, 21]
    X21 = np.ascontiguousarray(g.transpose(0, 3, 2, 1).reshape(NCORES, 21, F))
    x1h = np.zeros((NCORES, 86, 4096), np.float32)
    for q in range(4):
        r0 = 64 * (q % 2)
        c0 = 2048 * (q // 2)
        x1h[:, r0: r0 + 21, c0: c0 + 2048] = X21[:, :, 2048 * q: 2048 * (q + 1)]
        x1h[:, r0 + 21, c0: c0 + 2048] = 1.0
    P["x1_shards"] = x1h.astype(f16)
    return P


# ------------------------------------------------------------- device program
def _build_nc(debug=False, reps=1, trunc=9):
    import concourse.bacc as bacc
    import concourse.mybir as mybir
    import concourse.tile as tile
    from contextlib import ExitStack

    dt = mybir.dt
    f32 = dt.float32
    f16 = dt.float16
    AF = mybir.ActivationFunctionType
    ALU = mybir.AluOpType

    nc = bacc.Bacc()

    def din(name, shape, dt_=f16):
        return nc.declare_dram_parameter(name, list(shape), dt_, isOutput=False)

    x1_d = din("x1", (86, 4096))
    WALL_d = din("WALL", (128, _WTOT))
    CB_d = din("CB", (128, 4), f32)
    FCB_d = din("FCB", (16, 80, 2048))
    out_d = nc.declare_dram_parameter("out", [B, DOUT], f16, isOutput=True)
    dbg = {}
    if debug:
        for nm, p_ in [("dx2", 90), ("dx3", 128), ("dx4", 80), ("dx5", 128),
                       ("dx6", 128), ("dx7", 80)]:
            dbg[nm] = nc.declare_dram_parameter(nm, [p_, F], f32, isOutput=True)

    with tile.TileContext(nc) as tc, ExitStack() as ctx:
        wp = ctx.enter_context(tc.tile_pool(name="wts", bufs=1))
        ap_ = ctx.enter_context(tc.tile_pool(name="acts", bufs=1))
        pp = ctx.enter_context(tc.tile_pool(name="ps", bufs=5, space="PSUM"))
        ppcr = ctx.enter_context(tc.tile_pool(name="pcr", bufs=1, space="PSUM"))
        fcpp = ctx.enter_context(tc.tile_pool(name="fcps", bufs=1, space="PSUM"))
        fwp = ctx.enter_context(tc.tile_pool(name="fcw", bufs=8))
        stp = ctx.enter_context(tc.tile_pool(name="stp", bufs=4))
        sp = ctx.enter_context(tc.tile_pool(name="svals", bufs=1))

        WALL = wp.tile([128, _WTOT], f16, tag="WALL")
        nc.sync.dma_start(out=WALL[:, 0:90], in_=WALL_d[:, 0:90])
        x1q = []
        for q in range(4):
            t_ = ap_.tile([22, 2048], f16, tag=f"x1q{q}", name=f"x1q{q}")
            r0 = 64 * (q % 2)
            c0 = 2048 * (q // 2)
            if q == 0:
                nc.scalar.dma_start(
                    out=t_[:, 0:512], in_=x1_d[r0: r0 + 22, c0: c0 + 512])
                nc.scalar.dma_start(
                    out=t_[:, 512:2048],
                    in_=x1_d[r0: r0 + 22, c0 + 512: c0 + 2048])
            else:
                nc.scalar.dma_start(
                    out=t_[:, :], in_=x1_d[r0: r0 + 22, c0: c0 + 2048])
            x1q.append(t_)
        nc.sync.dma_start(out=WALL[:, 90:_WTOT], in_=WALL_d[:, 90:_WTOT])
        CB = wp.tile([128, 4], f32, tag="CB")
        nc.sync.dma_start(out=CB[:, :], in_=CB_d[:, :])

        def wv(nm):
            off, r, c = _WOFF[nm]
            return WALL[0:r, off: off + c]

        W1T = wv("W1T")
        W2M = wv("W2M")
        W3C, W3T = wv("W3C"), wv("W3T")
        CRW = (wv("CRW0"), wv("CRW1"))
        QWQ = (wv("QWQ0"), wv("QWQ1"))
        QWS = (wv("QWS0"), wv("QWS1"))
        D1W, D1T = wv("D1W"), wv("D1T")
        D2W, D2T = wv("D2W"), wv("D2T")
        FCBR = wv("FCBR")
        BC2 = CB[0:96, 0:1]
        BC3 = CB[0:80, 1:2]
        C2N = CB[0:128, 2:3]
        BD2 = CB[0:80, 3:4]

        ones = sp.tile([1, B], f16, tag="ones")
        nc.vector.memset(ones[:, :], 1.0)

        def mm(out, lhsT, rhs, start, stop=True):
            nc.tensor.matmul(out, lhsT, rhs, start=start, stop=stop)

        for _rep in range(reps):
            x2 = ap_.tile([90, F], f16, tag="x2")
            x3 = ap_.tile([128, F], f16, tag="x3")
            x3t = ap_.tile([32, F], f16, tag="x3t")
            x4 = ap_.tile([80, F], f16, tag="x4")
            em = ap_.tile([128, 2 * F], f16, tag="em")
            x5 = ap_.tile([128, F], f16, tag="x5")
            x6 = ap_.tile([128, F], f16, tag="x6")
            x7 = ap_.tile([80, F], f16, tag="x7")
            fws = [fwp.tile([80, 2048], f16, tag="fw", name=f"fw{_j}")
                   for _j in range(16)]
            fcp = fcpp.tile([B, DOUT], f32, tag="fcp")

            # head-halo boundary columns (block 0 reads t-block -1 = 0);
            # all other junk rows are covered by zero/ones weight columns
            nc.gpsimd.memset(x3[96:128, 0:B], 0.0)
            nc.gpsimd.memset(x3t[0:30, F - B: F], 0.0)
            nc.gpsimd.memset(x5[96:128, 0:B], 0.0)
            nc.gpsimd.memset(x6[96:128, 0:B], 0.0)

            def conv1(b):
                t = x1q[b // 4]
                c0 = (b % 4) * BW
                g0 = b * BW
                p1 = pp.tile([90, BW], f32, tag="ps", name="p1")
                mm(p1[:, :], W1T[:, :], t[:, c0: c0 + BW], True)
                nc.vector.tensor_relu(x2[:, g0: g0 + BW], p1[:, :])

            def conv2(b):
                g0 = b * BW
                p2 = pp.tile([96, BW], f32, tag="ps", name="p2")
                mm(p2[:, :], W2M[:, :], x2[:, g0: g0 + BW], True)
                nc.scalar.activation(
                    x3[0:96, g0: g0 + BW], p2[:, :], AF.Relu, bias=BC2)

            def x3cp(b):
                g0 = b * BW
                # head: prev block taus 5,6,7 (x3[64:94]) -> x3 rows 96:126
                lo = max(g0 - B, 0)
                off = lo - (g0 - B)
                nc.vector.tensor_copy(
                    x3[96:126, g0 + off: g0 + BW],
                    x3[64:94, lo: lo + BW - off])
                # tail: next block taus 0,1,2 -> x3t rows 0:30
                w = min(g0 + B + BW, F) - (g0 + B)
                nc.vector.tensor_copy(
                    x3t[0:30, g0: g0 + w], x3[0:30, g0 + B: g0 + B + w])

            def conv3(b):
                g0 = b * BW
                p3 = pp.tile([80, BW], f32, tag="ps", name="p3")
                mm(p3[:, :], W3C[:, :], x3[0:126, g0: g0 + BW], True,
                   stop=False)
                mm(p3[:, :], W3T[:, :], x3t[0:30, g0: g0 + BW], False)
                nc.scalar.activation(
                    x4[:, g0: g0 + BW], p3[:, :], AF.Tanh, bias=BC3)

            def vq_cr(b):
                g0 = b * BW
                pcr = ppcr.tile([128, 2 * BW], f32, tag="pcr", name="pcr")
                # the two halves land in different banks; each must be its
                # own start=True group (start clears has_written bank-wide,
                # and a start=False write would accumulate onto the
                # previous block's stale contents)
                mm(pcr[:, 0:BW], CRW[0][:, :], x4[:, g0: g0 + BW], True)
                mm(pcr[:, BW: 2 * BW], CRW[1][:, :], x4[:, g0: g0 + BW],
                   True)
                nc.scalar.activation(
                    em[:, 2 * g0: 2 * g0 + 2 * BW], pcr[:, :], AF.Exp,
                    bias=C2N, scale=2.0)

            def vq_q(b):
                g0 = b * BW
                e0 = em[:, 2 * g0: 2 * g0 + BW]
                e1 = em[:, 2 * g0 + BW: 2 * g0 + 2 * BW]
                pqq = pp.tile([96, BW], f32, tag="ps", name="pqq")
                mm(pqq[:, :], QWQ[0][:, :], e0, True, stop=False)
                mm(pqq[:, :], QWQ[1][:, :], e1, False)
                pqs = pp.tile([96, BW], f32, tag="ps", name="pqs")
                mm(pqs[:, :], QWS[0][:, :], e0, True, stop=False)
                mm(pqs[:, :], QWS[1][:, :], e1, False)
                st = stp.tile([96, BW], f32, tag="st", name="st")
                nc.vector.reciprocal_approx_fast(out=st[:, :], in_=pqs[:, :])
                with nc.allow_low_precision(reason="softmax 1/s in fp16"):
                    nc.vector.tensor_tensor(
                        x5[0:96, g0: g0 + BW], pqq[:, :], st[:, :],
                        ALU.mult)

            def x5cp(b):
                g0 = b * BW
                lo = max(g0 - B, 0)
                off = lo - (g0 - B)
                nc.vector.tensor_copy(
                    x5[96:116, g0 + off: g0 + BW],
                    x5[64:84, lo: lo + BW - off])

            def d1(b):
                g0 = b * BW
                pd1 = pp.tile([96, BW], f32, tag="ps", name="pd1")
                mm(pd1[:, :], D1W[:, :], x5[0:116, g0: g0 + BW], True,
                   stop=False)
                w = min(g0 + B + BW, F) - (g0 + B)
                mm(pd1[:, 0:w], D1T[:, :], x5[0:20, g0 + B: g0 + B + w],
                   False)
                nc.vector.tensor_relu(x6[0:96, g0: g0 + BW], pd1[:, :])

            def x6cp(b):
                g0 = b * BW
                lo = max(g0 - B, 0)
                off = lo - (g0 - B)
                nc.vector.tensor_copy(
                    x6[96:116, g0 + off: g0 + BW],
                    x6[64:84, lo: lo + BW - off])

            def d2(b):
                g0 = b * BW
                pd2 = pp.tile([80, BW], f32, tag="ps", name="pd2")
                mm(pd2[:, :], D2W[:, :], x6[0:116, g0: g0 + BW], True,
                   stop=False)
                w = min(g0 + B + BW, F) - (g0 + B)
                mm(pd2[:, 0:w], D2T[:, :], x6[0:20, g0 + B: g0 + B + w],
                   False)
                nc.scalar.activation(
                    x7[:, g0: g0 + BW], pd2[:, :], AF.Relu, bias=BD2)

            def fc(q):
                if q == 0:
                    mm(fcp[:, :], ones[0:1, 0:B], FCBR[0:1, :], True,
                       stop=False)
                for k in range(4):
                    tb = 4 * q + k
                    mm(fcp[:, :], x7[:, tb * B: (tb + 1) * B],
                       fws[q][:, k * 512: (k + 1) * 512],
                       False, stop=(tb == NTB - 1))

            # fused software pipeline; lags satisfy cross-block halo deps
            # (b+1) and the psum pool rotation (bufs=5, 6 allocs/iter)
            for i in range(NB + 10):
                if i < NB:
                    conv1(i)
                if i < 16 and trunc >= 9:
                    nc.sync.dma_start(out=fws[i][:, :], in_=FCB_d[i, :, :])
                if 0 <= i - 1 < NB and trunc >= 2:
                    conv2(i - 1)
                if 0 <= i - 2 < NB and trunc >= 3:
                    x3cp(i - 2)
                if 0 <= i - 3 < NB and trunc >= 3:
                    conv3(i - 3)
                if 0 <= i - 4 < NB and trunc >= 4:
                    vq_cr(i - 4)
                if 0 <= i - 5 < NB and trunc >= 5:
                    vq_q(i - 5)
                if 0 <= i - 6 < NB and trunc >= 6:
                    x5cp(i - 6)
                if 0 <= i - 7 < NB and trunc >= 7:
                    d1(i - 7)
                if 0 <= i - 8 < NB and trunc >= 8:
                    x6cp(i - 8)
                if 0 <= i - 9 < NB and trunc >= 8:
                    d2(i - 9)
                if 0 <= i - 10 < NB and trunc >= 9:
                    fc(i - 10)

            if debug:
                for nm, t in [("dx2", x2), ("dx3", x3), ("dx4", x4),
                              ("dx5", x5), ("dx6", x6), ("dx7", x7)]:
                    nc.gpsimd.dma_start(out=dbg[nm][:, :], in_=t[:, 0:F])

            out_sb = sp.tile([B, DOUT], f16, tag="out")
            if trunc >= 9:
                nc.scalar.activation(out_sb[:, :], fcp[:, :], AF.Tanh)
            else:
                tdump = {1: x2, 2: x3, 3: x4, 4: em, 5: x5, 6: x5,
                         7: x6, 8: x7}[trunc]
                nc.scalar.activation(out_sb[0:64, :],
                                     tdump[0:64, 0:512], AF.Copy)
            nc.sync.dma_start(out=out_d[:, :], in_=out_sb[:, :])

    nc.compile()
    return nc


def _get_nc():
    if "nc" not in _CACHE:
        _CACHE["nc"] = _build_nc()
    return _CACHE["nc"]


_COMMON = ("WALL", "CB", "FCB")


def kernel(**inputs):
    P = _host_prep(**inputs)
    nc = _get_nc()
    common = {k: P[k] for k in _COMMON}
    in_maps = [dict(common, x1=P["x1_shards"][i]) for i in range(NCORES)]
    from concourse.bass_utils import run_bass_kernel_spmd

    res = run_bass_kernel_spmd(nc, in_maps, list(range(NCORES)))
    return np.concatenate([res.results[i]["out"] for i in range(NCORES)],
                          axis=0).astype(np.float32)


if __name__ == "__main__":
    import reference

    inputs = {k: np.asarray(v) for k, v in reference.setup_inputs().items()}
    out = kernel(**inputs)
    exp = np.asarray(reference.reference(**inputs))
    err = np.abs(out - exp).max() / (np.abs(exp).max() + 1e-30)
    print("Relative error:", err)
